# revision 1
# baseline (speedup 1.0000x reference)
"""Trainium2 Bass kernel for nn_AgeUGP_v2 (gnn_message_passing).

Reference pipeline:
  snp_h[b,n,f] = snp[b,n] * filters[f,n]
  gathered     = snp_h[:, snp_ids, :]
  per_gene     = segment_sum(gathered, node_seg)   # node_seg sorted
  sample_h     = per_gene.mean(-1)
  h1 = sample_h @ W1 ... tiny MLP tail

Algebraic collapse: the filter axis F is only averaged at the end, so
  sample_h[b,g] = sum_{i in seg g} snp[b, id_i] * fbar[id_i],
  fbar = mean(filters, axis=0).

Device strategy (8 NeuronCores, genes sharded across cores):
  - SNP axis padded to 64 chunks of 8192.  4 table phases; in phase T the
    128 partitions hold (chunk, batch) tables of v = snp * fbar in f32:
    partition p = 16g + 8h + b holds chunk 16T + g + 8h, batch b.
    Tables are built in-place from a host-permuted copy of snp (wide
    quarter DMAs); fbar is produced fused: a host-permuted bf16 copy of
    filters (rows on partitions) is hit with a single 1/8-valued
    mean+replicate PE matmul per 512 columns, whose PSUM output directly
    multiplies the table on DVE.
  - One merged pass per table: gpsimd ap_gather gathers the nodes of both
    chunk halves in one gene-ordered stream (group g's shared index stream
    is applied to all 16 lanes; each node is valid on its half's 8 lanes,
    junk elsewhere is excluded by the combine).  A DVE tensor_tensor_scan
    (fp32, in place over the gather buffer) forms prefix sums; a second
    ap_gather extracts prefixes at the A-end and B-end gene boundaries;
    one adjacent-difference gives per-(half,gene,batch) partials with no
    padding inflation and no masking.  (ap_gather index APs must start at
    a tile base: sliced index APs silently misread on HW.)
  - Per pass, PE matmuls against 0/1 lane-selection columns (selA for the
    A-half lanes, selB for B) form the valid-lane sums in PSUM; DVE
    accumulates into sample_h [gene, batch].
  - PE matmul with the core's W1 shard (bf16, host-permuted for wide
    loads) -> partial h1 [8, 1024].
  - host sums the 8 partials and runs the tiny MLP tail (0.01% of FLOPs).
Emission is software-pipelined (gather p+1 ahead of pass-p tail; tables
double-buffered, snp/filter loads interleaved at quarter granularity) so
Pool/DVE/DMA overlap at ~23.5us per table cycle each.
"""

import numpy as np

B = 8
N_SNPS = 500000
N_NODES = 2000000
N_GENES = 20000
N_FILT = 8
N_CORES = 8
BN_EPS = 1e-5

_P = 128
_NCHUNK = 64  # SNP chunks
_NTAB = 4  # table phases
_NPASS = 4  # gather passes (one per table; chunk halves merged)
_EPAD = 16


def make_cfg(n_snps, n_genes, n_cores, chunk, d1, J, qchunks):
    snp_pad = _NCHUNK * chunk
    piece = snp_pad // _P
    assert snp_pad >= n_snps
    assert J % 16 == 0
    gpc = n_genes // n_cores
    jt = -(-gpc // _P)
    gpad_ = jt * _P
    ns = gpad_ + gpc + 1  # boundaries: dummy + gpad A-ends + gpc B-ends
    nspad = -(-ns // _EPAD) * _EPAD
    return dict(
        n_snps=n_snps, snp_pad=snp_pad, chunk=chunk, piece=piece,
        n_genes=n_genes, n_cores=n_cores, gpc=gpc, gpad=jt * _P, jt=jt,
        d1=d1, J=J, qchunks=qchunks, ns=ns, nspad=nspad,
    )


def full_cfg(J):
    return make_cfg(N_SNPS, N_GENES, N_CORES, 8192, 1024, J, 16)


# ---------------------------------------------------------------- device program
def build_program(cfg):
    import concourse.bass as bass
    import concourse.bacc as bacc
    import concourse.mybir as mybir
    import concourse.tile as tile

    fp32 = mybir.dt.float32
    bf16 = mybir.dt.bfloat16
    i16 = mybir.dt.int16

    chunk, piece, snp_pad = cfg["chunk"], cfg["piece"], cfg["snp_pad"]
    jt, d1, J = cfg["jt"], cfg["d1"], cfg["J"]
    gpad, nspad, gpc = cfg["gpad"], cfg["nspad"], cfg["gpc"]

    nc = bacc.Bacc(
        "TRN2", target_bir_lowering=False, debug=False, num_devices=cfg["n_cores"]
    )

    tp = snp_pad // (_NTAB * _P)  # fbar T-slice columns per partition
    n_sp = chunk // tp  # routing matrices (shared across T)
    assert 16 * n_sp == _P
    qf = 4 if tp % 4 == 0 else 1  # fbar sub-loads per T-slice
    assert tp % qf == 0

    snp_in = nc.dram_tensor(
        "snp_perm", [_P, _NTAB * chunk], fp32, kind="ExternalInput"
    )
    filt_in = nc.dram_tensor(
        "filt_perm", [_P, _NTAB * chunk], bf16, kind="ExternalInput"
    )
    gidx_in = nc.dram_tensor(
        "gidx", [_P, _NPASS * (J // 16)], i16, kind="ExternalInput"
    )
    eidx_in = nc.dram_tensor(
        "eidx", [_P, _NPASS * (nspad // 16)], i16, kind="ExternalInput"
    )
    sel_in = nc.dram_tensor("sel", [_P, 16], bf16, kind="ExternalInput")
    route_in = nc.dram_tensor("mroute", [_P, _P], bf16, kind="ExternalInput")
    w1_in = nc.dram_tensor("w1c", [_P, jt * d1], bf16, kind="ExternalInput")
    h1_out = nc.dram_tensor("h1p", [B, d1], fp32, kind="ExternalOutput")

    with tile.TileContext(nc) as tc:
        with (
            tc.tile_pool(name="per", bufs=1) as perpool,
            tc.tile_pool(name="tab", bufs=2) as tabpool,
            tc.tile_pool(name="fbr", bufs=1) as fbrpool,
            tc.tile_pool(name="gs", bufs=2) as gspool,
            tc.tile_pool(name="ft", bufs=2) as ftpool,
            tc.tile_pool(name="ex", bufs=1) as expool,
            tc.tile_pool(name="dd", bufs=1) as ddpool,
            tc.tile_pool(name="w1", bufs=3) as w1pool,
            tc.tile_pool(name="ps", bufs=4, space="PSUM") as pspool,
            tc.tile_pool(name="psw", bufs=1, space="PSUM") as pswpool,
            tc.tile_pool(name="psh", bufs=2, space="PSUM") as pshpool,
        ):
            # mean+replication routing matrix
            route = perpool.tile([_P, _P], bf16, tag="route")
            nc.sync.dma_start(route[:], route_in.ap())
            sel = perpool.tile([_P, 16], bf16, tag="sel")
            nc.sync.dma_start(sel[:], sel_in.ap())
            zs = perpool.tile([_P, 1], fp32, tag="zs")
            nc.vector.memset(zs[:], 0.0)

            # SBUF accumulator for sample_h [gene-tile, (t, b)]
            sh = perpool.tile([_P, jt * B], fp32, tag="sh")
            nc.vector.memset(sh[:], 0.0)

            vtabs = {}
            rc = min(512, chunk)
            nblk = chunk // rc
            nhv = 4 if nblk % 4 == 0 else 1
            fhalf = chunk // nhv

            def emit_table(T):
                # filters T-slice (rows on partitions via host perm), cast to
                # bf16 per block; one mean+replicate matmul per 512 columns
                vtab = tabpool.tile([_P, chunk], fp32, tag="vtab", name=f"vtab{T}")
                for hv in range(nhv):
                    # interleave snp/filter quarter-loads so multiply blocks
                    # start as early as possible
                    nc.sync.dma_start(
                        vtab[:, hv * fhalf : (hv + 1) * fhalf],
                        snp_in.ap()[:, T * chunk + hv * fhalf :
                                    T * chunk + (hv + 1) * fhalf],
                    )
                    ft = ftpool.tile(
                        [_P, fhalf], bf16, tag="ftl", name=f"ftl{T}_{hv}"
                    )
                    nc.sync.dma_start(
                        ft[:],
                        filt_in.ap()[:, T * chunk + hv * fhalf :
                                     T * chunk + (hv + 1) * fhalf],
                    )
                    for blk in range(nblk // nhv):
                        pr = pspool.tile([_P, rc], fp32, tag="pr", name="pr")
                        nc.tensor.matmul(
                            pr[:], route[:], ft[:, blk * rc : (blk + 1) * rc],
                            start=True, stop=True,
                        )
                        ks = slice(hv * fhalf + blk * rc,
                                   hv * fhalf + (blk + 1) * rc)
                        nc.vector.tensor_mul(vtab[:, ks], vtab[:, ks], pr[:])
                vtabs[T] = vtab

            def emit_gather(pidx):
                gidx = gspool.tile(
                    [_P, J // 16], i16, tag="gidx", name=f"gidx{pidx}"
                )
                nc.sync.dma_start(
                    gidx[:],
                    gidx_in.ap()[:, pidx * (J // 16) : (pidx + 1) * (J // 16)],
                )
                gout = gspool.tile([_P, J], fp32, tag="gout", name=f"gout{pidx}")
                nc.gpsimd.ap_gather(
                    gout[:], vtabs[pidx][:], gidx[:],
                    channels=_P, num_elems=chunk, d=1, num_idxs=J,
                )
                return gout

            def emit_tail(pidx, gout):
                # in-place prefix scan: safe, the scan never reads its output
                q = gout
                zbc = bass.AP(zs.tensor, zs[:].offset, [zs[:].ap[0], [0, J]])
                nc.vector.tensor_tensor_scan(
                    q[:], zbc, gout[:], 0.0,
                    op0=mybir.AluOpType.add, op1=mybir.AluOpType.add,
                )
                eidx = gspool.tile(
                    [_P, nspad // 16], i16, tag="eidx", name=f"eidx{pidx}"
                )
                nc.sync.dma_start(
                    eidx[:],
                    eidx_in.ap()[:, pidx * (nspad // 16) : (pidx + 1) * (nspad // 16)],
                )
                ex = expool.tile([_P, nspad], fp32, tag="ex", name=f"ex{pidx}")
                nc.gpsimd.ap_gather(
                    ex[:], q[:], eidx[:],
                    channels=_P, num_elems=J, d=1, num_idxs=nspad,
                )
                # E = [Q0, A-ends (gpad, padded), B-ends (gpc)]; adjacent
                # diffs give ddA at [0,gpad) and ddB at [gpad, gpad+gpc)
                nd = gpad + gpc
                dd = ddpool.tile([_P, 2 * gpad], bf16, tag="dd", name=f"dd{pidx}")
                if 2 * gpad > nd:
                    nc.vector.memset(dd[:, nd:], 0.0)
                nc.vector.tensor_sub(dd[:, :nd], ex[:, 1 : nd + 1], ex[:, :nd])
                pst = pshpool.tile([_P, jt * B], fp32, tag="pst", name="pst")
                for t in range(jt):
                    nc.tensor.matmul(
                        pst[:, t * B : (t + 1) * B],
                        dd[:, t * _P : (t + 1) * _P],
                        sel[:, :8],
                        start=True, stop=False,
                    )
                    nc.tensor.matmul(
                        pst[:, t * B : (t + 1) * B],
                        dd[:, gpad + t * _P : gpad + (t + 1) * _P],
                        sel[:, 8:],
                        start=False, stop=True,
                    )
                nc.vector.tensor_add(sh[:], sh[:], pst[:])

            # software-pipelined emission: gather(p+1) ahead of tail(p)
            emit_table(0)
            gouts = {0: emit_gather(0)}
            for p in range(_NPASS):
                if p + 1 < _NTAB:
                    emit_table(p + 1)
                if p + 1 < _NPASS:
                    gouts[p + 1] = emit_gather(p + 1)
                emit_tail(p, gouts.pop(p))

            shb = perpool.tile([_P, jt * B], bf16, tag="shb")
            nc.vector.tensor_copy(shb[:], sh[:])

            # ---- W1 matmul: accumulate over jt K-tiles --------------------
            n_half = min(512, d1)
            n_banks = -(-d1 // n_half)
            pss = []
            for nb in range(n_banks):
                pst = pswpool.tile([_P, n_half], fp32, tag=f"ps{nb}", name=f"ps{nb}")
                pss.append(pst)
            wgrp = 5 if jt % 5 == 0 else 1  # K-tiles per W1 load
            for jg in range(jt // wgrp):
                w1t = w1pool.tile([_P, wgrp * d1], bf16, tag="w1t")
                nc.sync.dma_start(
                    w1t[:],
                    w1_in.ap()[:, jg * wgrp * d1 : (jg + 1) * wgrp * d1],
                )
                for jl in range(wgrp):
                    j = jg * wgrp + jl
                    lhsT = shb[:, j * B : (j + 1) * B]
                    for nb in range(n_banks):
                        nc.tensor.matmul(
                            pss[nb][:B, :],
                            lhsT,
                            w1t[:, jl * d1 + nb * n_half : jl * d1 + (nb + 1) * n_half],
                            start=(j == 0),
                            stop=(j == jt - 1),
                        )

            h1 = perpool.tile([B, d1], fp32, tag="h1")
            for nb in range(n_banks):
                nc.vector.tensor_copy(
                    h1[:, nb * n_half : (nb + 1) * n_half], pss[nb][:B, :]
                )
            nc.sync.dma_start(h1_out.ap(), h1[:])

    nc.compile()
    return nc


# ---------------------------------------------------------------- host side
def _wrap16(streams):
    """[8, J] per-group streams -> [128, J//16] wrapped-16 layout."""
    ngrp, J = streams.shape
    assert ngrp == 8 and J % 16 == 0
    out = np.zeros((_P, J // 16), streams.dtype)
    for g in range(8):
        out[g * 16 : (g + 1) * 16, :] = streams[g].reshape(J // 16, 16).T
    return out


def prep_inputs(cfg, snp, snp_ids, node_seg, filters, W1):
    """Index/metadata preprocessing + zero-padding + pure layout permutation;
    all value computation happens on device."""
    import ml_dtypes

    snp_pad_n, chunk, piece = cfg["snp_pad"], cfg["chunk"], cfg["piece"]
    gpc, gpad, d1 = cfg["gpc"], cfg["gpad"], cfg["d1"]
    n_genes, n_snps = cfg["n_genes"], cfg["n_snps"]
    J, nspad = cfg["J"], cfg["nspad"]
    n_cores = cfg["n_cores"]
    ppc = chunk // piece

    snp_p = np.zeros((B, snp_pad_n), np.float32)
    snp_p[:, :n_snps] = np.asarray(snp, np.float32)
    filt_p = np.zeros((B, snp_pad_n), np.float32)
    filt_p[:, :n_snps] = np.asarray(filters, np.float32)
    # filt_perm[q, T*chunk + k] = filters[q%8, (16T + q//8)*chunk + k]
    filt_perm = np.empty((_P, _NTAB * chunk), np.float32)  # cast below
    for T in range(_NTAB):
        view = filt_p[:, 16 * T * chunk : (16 * T + 16) * chunk].reshape(
            B, 16, chunk
        )  # [r, sp, k]
        filt_perm[:, T * chunk : (T + 1) * chunk] = (
            view.transpose(1, 0, 2).reshape(_P, chunk)
        )
    filt_perm_bf = filt_perm.astype(ml_dtypes.bfloat16)

    # pure layout permutation: row 16g+8h+b, cols [T*chunk,(T+1)*chunk) holds
    # snp[b, (16T+g+8h)*chunk : +chunk]
    snp_perm = np.empty((_P, _NTAB * chunk), np.float32)
    for T in range(_NTAB):
        view = snp_p[:, 16 * T * chunk : (16 * T + 16) * chunk].reshape(
            B, 2, 8, chunk
        )  # [b, h, g, k]
        snp_perm[:, T * chunk : (T + 1) * chunk] = (
            view.transpose(2, 1, 0, 3).reshape(_P, chunk)
        )

    # mean+replicate routing: out[m, j] = (1/8) sum_r filters[r, c(m)*chunk+j]
    # lhsT[q, m] = 1/8 iff q//8 == g(m) + 8*h(m)
    mroute = np.zeros((_P, _P), ml_dtypes.bfloat16)
    m = np.arange(_P)
    g, hb = m // 16, m % 16
    hh = hb // 8
    mroute[:, :] = 0
    for mm in range(_P):
        spt = g[mm] + 8 * hh[mm]
        mroute[spt * 8 : spt * 8 + 8, mm] = 1.0 / N_FILT

    sel = np.zeros((_P, 16), ml_dtypes.bfloat16)
    for p in range(_P):
        sel[p, p % 16] = 1.0

    ids = np.asarray(snp_ids).astype(np.int64)
    seg = np.asarray(node_seg).astype(np.int64)
    gene_starts = np.searchsorted(seg, np.arange(0, n_genes + 1))
    node_chunk = ids // chunk
    node_lidx = (ids % chunk).astype(np.int16)

    W1f = np.asarray(W1, np.float32)
    per_core = []
    for c in range(n_cores):
        lo, hi = gene_starts[c * gpc], gene_starts[(c + 1) * gpc]
        cid_chunk = node_chunk[lo:hi]
        cid_lidx = node_lidx[lo:hi]
        cid_gene = seg[lo:hi] - c * gpc  # local gene, sorted ascending

        gidx = np.zeros((_NPASS, 8, J), np.int16)
        eidx = np.zeros((_NPASS, 8, nspad), np.int16)
        for T in range(_NTAB):
            for g_ in range(8):
                chA, chB = 16 * T + g_, 16 * T + 8 + g_
                mA = cid_chunk == chA
                mB = cid_chunk == chB
                lidxA, lgeneA = cid_lidx[mA], cid_gene[mA]
                lidxB, lgeneB = cid_lidx[mB], cid_gene[mB]
                cntA, cntB = len(lidxA), len(lidxB)
                assert cntA + cntB + 1 <= J, f"bucket {cntA+cntB} exceeds J={J}"
                # merged stream: [dummy, chunk-A nodes by gene, chunk-B nodes]
                gidx[T, g_, 1 : 1 + cntA] = lidxA
                gidx[T, g_, 1 + cntA : 1 + cntA + cntB] = lidxB
                # boundary positions: [0, A-ends (gpad, pad=end-of-A), B-ends]
                FA = np.searchsorted(lgeneA, np.arange(1, gpc + 1))
                FB = cntA + np.searchsorted(lgeneB, np.arange(1, gpc + 1))
                pos = np.zeros(nspad, np.int64)
                pos[1 : 1 + gpc] = FA
                pos[1 + gpc : 1 + gpad] = FA[-1]
                pos[1 + gpad : 1 + gpad + gpc] = FB
                pos[1 + gpad + gpc :] = FB[-1]
                eidx[T, g_] = pos.astype(np.int16)

        w1c = np.zeros((gpad, d1), np.float32)
        w1c[:gpc] = W1f[c * gpc : (c + 1) * gpc]
        jt_ = gpad // _P
        # w1 perm: [p, j*d1 + col] = w1c[j*128 + p, col]
        w1perm = np.ascontiguousarray(
            w1c.reshape(jt_, _P, d1).transpose(1, 0, 2).reshape(_P, jt_ * d1)
        ).astype(ml_dtypes.bfloat16)
        gidx_all = np.concatenate(
            [_wrap16(gidx[p]) for p in range(_NPASS)], axis=1
        )
        eidx_all = np.concatenate(
            [_wrap16(eidx[p]) for p in range(_NPASS)], axis=1
        )
        core_map = dict(
            snp_perm=snp_perm, filt_perm=filt_perm_bf, sel=sel, w1c=w1perm,
            mroute=mroute, gidx=gidx_all, eidx=eidx_all,
        )
        per_core.append(core_map)
    return per_core


def host_tail(h1_sum, b1, g1, be1, W2, b2, g2, be2, W3, b3, g3, be3,
              Wh1, bh1, gh, beh, Wh2, bh2):
    def bn(x, g, be):
        return x * (g / np.sqrt(np.float32(1.0 + BN_EPS))) + be

    relu = lambda x: np.maximum(x, np.float32(0.0))
    h = relu(bn(h1_sum + b1, g1, be1))
    h = relu(bn(h @ W2 + b2, g2, be2))
    feat = relu(bn(h @ W3 + b3, g3, be3))
    m = relu(bn(feat[:, :15] @ Wh1 + bh1, gh, beh))
    return (m @ Wh2 + bh2).astype(np.float32)


def pick_J(snp_ids, node_seg, chunk=8192):
    ids = np.asarray(snp_ids).astype(np.int64)
    seg = np.asarray(node_seg).astype(np.int64)
    gpc = N_GENES // N_CORES
    gene_starts = np.searchsorted(seg, np.arange(0, N_GENES + 1, gpc))
    mx = 0
    for c in range(N_CORES):
        lo, hi = gene_starts[c], gene_starts[c + 1]
        cnt = np.bincount(ids[lo:hi] // chunk, minlength=_NCHUNK)
        comb = cnt.reshape(_NTAB, 2, 8).sum(axis=1)  # chunk + chunk+8 merged
        mx = max(mx, int(comb.max()))
    J = -(-(mx + 1) // 16) * 16
    # int16 stream/boundary indices: fail loudly rather than wrap silently
    assert J <= 32752, f"pass stream length {J} exceeds int16 index range"
    return J


_CACHE = {}


def kernel(snp, snp_ids, node_seg, filters, W1, b1, g1, be1, W2, b2, g2, be2,
           W3, b3, g3, be3, Wh1, bh1, gh, beh, Wh2, bh2):
    from concourse import bass_utils

    J = pick_J(snp_ids, node_seg)
    cfg = full_cfg(J)

    key = ("full", J)
    if key not in _CACHE:
        _CACHE[key] = build_program(cfg)
    nc = _CACHE[key]

    in_maps = prep_inputs(cfg, snp, snp_ids, node_seg, filters, W1)
    res = bass_utils.run_bass_kernel_spmd(
        nc, in_maps, core_ids=list(range(cfg["n_cores"]))
    )
    h1_sum = np.zeros((B, cfg["d1"]), np.float32)
    for c in range(cfg["n_cores"]):
        h1_sum += res.results[c]["h1p"]

    f32 = lambda x: np.asarray(x, np.float32)
    return host_tail(h1_sum, f32(b1), f32(g1), f32(be1), f32(W2), f32(b2),
                     f32(g2), f32(be2), f32(W3), f32(b3), f32(g3), f32(be3),
                     f32(Wh1), f32(bh1), f32(gh), f32(beh), f32(Wh2), f32(bh2))



# revision 2
# speedup vs baseline: 1.0905x; 1.0905x over previous
"""Trainium2 Bass kernel for nn_AgeUGP_v2 (gnn_message_passing).

Reference pipeline:
  snp_h[b,n,f] = snp[b,n] * filters[f,n]
  gathered     = snp_h[:, snp_ids, :]
  per_gene     = segment_sum(gathered, node_seg)   # node_seg sorted
  sample_h     = per_gene.mean(-1)
  h1 = sample_h @ W1 ... tiny MLP tail

Algebraic collapse: the filter axis F is only averaged at the end, so
  sample_h[b,g] = sum_{i in seg g} snp[b, id_i] * fbar[id_i],
  fbar = mean(filters, axis=0).

Device strategy v2 (8 NeuronCores, genes sharded across cores):
  - Per-core SNP COMPACTION: each core's nodes reference ~197k unique SNPs
    (of 500k); the host selects and orders just those (pure permutation),
    split into 64 chunks of Kc.  4 table passes; pass T holds 16 chunks on
    128 partitions: partition p = 16g + 8h + b carries chunk 16T+g+8h,
    batch b.
  - ZERO-JUNK split tables: each partition's gather table is [2*Kc+2] with
    its chunk's values v = snp * fbar at [h*Kc : (h+1)*Kc] and ZEROS
    elsewhere (zeroed once per buffer; DMAs only rewrite data halves).
    An index in [0,Kc) reads chunk A's value on h=0 lanes and exact 0 on
    h=1 lanes (and vice versa), so the 16-lane shared-index junk vanishes
    arithmetically: A/B contributions merge into ONE gene segment.
  - fbar is produced fused on device: a bf16 host-permuted copy of filters
    is hit with 1/8-valued mean+replicate PE matmuls (routeA/routeB) whose
    PSUM output multiplies the table halves on DVE (zeros stay zero).
  - One gpsimd ap_gather per pass streams both chunks' nodes gene-ordered
    (per-gene counts padded to EVEN with pads pointing at the zero column).
    A DVE tensor_tensor_scan with data0/data1 = even/odd stride-2 views
    forms PAIR prefix sums in place (halving scan and extraction size); a
    second ap_gather extracts one prefix per gene END; one adjacent
    difference gives per-(gene,half,batch) sums; a single sel matmul per
    gene tile folds halves+lanes into sample_h [gene, batch] in PSUM.
  - PE matmul with the core's W1 shard (bf16) -> partial h1 [8, 1024];
    host sums the 8 partials and runs the tiny MLP tail (0.01% of FLOPs).
Emission is software-pipelined (gather p+1 ahead of pass-p tail; tables
manually double-buffered so the zero halves persist across passes).
"""

import numpy as np

B = 8
N_SNPS = 500000
N_NODES = 2000000
N_GENES = 20000
N_FILT = 8
N_CORES = 8
BN_EPS = 1e-5

_P = 128
_NCHUNK = 64  # compact SNP chunks per core
_NTAB = 4  # table passes
_EPAD = 16


def make_cfg(Kc, J, n_genes=N_GENES, n_cores=N_CORES, d1=1024):
    gpc = n_genes // n_cores
    jt = -(-gpc // _P)
    gpad = jt * _P
    ns = gpc + 1  # boundaries: dummy zero + one end per gene
    nspad = -(-ns // _EPAD) * _EPAD
    assert J % 16 == 0 and J % 4 == 0
    assert 2 * Kc + 2 <= 2**15, "gather table exceeds num_elems limit"
    assert J <= 32752, "stream length exceeds int16 index range"
    return dict(
        Kc=Kc, J=J, gpc=gpc, gpad=gpad, jt=jt, d1=d1, ns=ns, nspad=nspad,
        n_cores=n_cores,
    )


# ---------------------------------------------------------------- device program
def build_program(cfg):
    import concourse.bass as bass
    import concourse.bacc as bacc
    import concourse.mybir as mybir
    import concourse.tile as tile

    fp32 = mybir.dt.float32
    bf16 = mybir.dt.bfloat16
    i16 = mybir.dt.int16

    Kc, J = cfg["Kc"], cfg["J"]
    jt, d1 = cfg["jt"], cfg["d1"]
    gpc, gpad, nspad = cfg["gpc"], cfg["gpad"], cfg["nspad"]
    TW = 2 * Kc + 2  # table width: [A-half | B-half | zero col pair]
    JH = J // 2

    nc = bacc.Bacc(
        "TRN2", target_bir_lowering=False, debug=False, num_devices=cfg["n_cores"]
    )

    snp_in = nc.dram_tensor("snp_perm", [_P, _NTAB * Kc], fp32, kind="ExternalInput")
    filt_in = nc.dram_tensor("filt_perm", [_P, _NTAB * Kc], bf16, kind="ExternalInput")
    gidx_in = nc.dram_tensor("gidx", [_P, _NTAB * (J // 16)], i16, kind="ExternalInput")
    eidx_in = nc.dram_tensor(
        "eidx", [_P, _NTAB * (nspad // 16)], i16, kind="ExternalInput"
    )
    sel_in = nc.dram_tensor("sel", [_P, 8], bf16, kind="ExternalInput")
    route_in = nc.dram_tensor("mroute", [_P, 2 * _P], bf16, kind="ExternalInput")
    w1_in = nc.dram_tensor("w1c", [_P, jt * d1], bf16, kind="ExternalInput")
    h1_out = nc.dram_tensor("h1p", [B, d1], fp32, kind="ExternalOutput")

    rc = Kc // 8  # route/mul block width (Kc is 16-aligned; rc*8 == Kc)
    assert rc * 8 == Kc and rc <= 512

    with tile.TileContext(nc) as tc:
        with (
            tc.tile_pool(name="per", bufs=1) as perpool,
            tc.tile_pool(name="gs", bufs=2) as gspool,
            tc.tile_pool(name="ft", bufs=2) as ftpool,
            tc.tile_pool(name="ex", bufs=2) as expool,
            tc.tile_pool(name="w1", bufs=2) as w1pool,
            tc.tile_pool(name="ps", bufs=4, space="PSUM") as pspool,
            tc.tile_pool(name="psw", bufs=1, space="PSUM") as pswpool,
            tc.tile_pool(name="psh", bufs=2, space="PSUM") as pshpool,
        ):
            route = perpool.tile([_P, 2 * _P], bf16, tag="route")
            nc.sync.dma_start(route[:], route_in.ap())
            sel8 = perpool.tile([_P, 8], bf16, tag="sel8")
            nc.sync.dma_start(sel8[:], sel_in.ap())

            # sample_h accumulator [gene-tile, (t, b)]
            sh = perpool.tile([_P, jt * B], fp32, tag="sh")
            nc.vector.memset(sh[:], 0.0)
            # dd holds per-(lane,gene) sums; pad cols stay zero forever
            dd = perpool.tile([_P, gpad], bf16, tag="dd")
            nc.vector.memset(dd[:], 0.0)

            # manually double-buffered tables; zero halves persist, DMAs
            # only rewrite data halves each pass (gpsimd memsets: Pool is
            # idle during the lead-in)
            vt = []
            for i in range(2):
                v = perpool.tile([_P, TW], fp32, tag=f"vtab{i}")
                nc.gpsimd.memset(v[:], 0.0)
                vt.append(v)

            def emit_table(T):
                vtab = vt[T % 2]
                ft = ftpool.tile([_P, Kc], bf16, tag="ftl", name=f"ftl{T}")
                nc.sync.dma_start(ft[:], filt_in.ap()[:, T * Kc : (T + 1) * Kc])
                for g in range(8):
                    r0 = 16 * g
                    nc.sync.dma_start(
                        vtab[r0 : r0 + 8, 0:Kc],
                        snp_in.ap()[r0 : r0 + 8, T * Kc : (T + 1) * Kc],
                    )
                    nc.sync.dma_start(
                        vtab[r0 + 8 : r0 + 16, Kc : 2 * Kc],
                        snp_in.ap()[r0 + 8 : r0 + 16, T * Kc : (T + 1) * Kc],
                    )
                for blk in range(8):
                    ks = slice(blk * rc, (blk + 1) * rc)
                    prA = pspool.tile([_P, rc], fp32, tag="pr", name="prA")
                    nc.tensor.matmul(
                        prA[:], route[:, 0:_P], ft[:, ks], start=True, stop=True
                    )
                    nc.vector.tensor_mul(vtab[:, ks], vtab[:, ks], prA[:])
                    prB = pspool.tile([_P, rc], fp32, tag="pr", name="prB")
                    nc.tensor.matmul(
                        prB[:], route[:, _P : 2 * _P], ft[:, ks], start=True, stop=True
                    )
                    ksB = slice(Kc + blk * rc, Kc + (blk + 1) * rc)
                    nc.vector.tensor_mul(vtab[:, ksB], vtab[:, ksB], prB[:])

            def emit_gather(p):
                gidx = gspool.tile([_P, J // 16], i16, tag="gidx", name=f"gidx{p}")
                nc.sync.dma_start(
                    gidx[:], gidx_in.ap()[:, p * (J // 16) : (p + 1) * (J // 16)]
                )
                gout = gspool.tile([_P, J], fp32, tag="gout", name=f"gout{p}")
                nc.gpsimd.ap_gather(
                    gout[:], vt[p % 2][:], gidx[:],
                    channels=_P, num_elems=TW, d=1, num_idxs=J,
                )
                return gout

            def emit_tail(p, gout):
                # pair prefix scan, in place into the first half (writes
                # trail the stride-2 reads)
                gall = gout[:]
                even = bass.AP(gall.tensor, gall.offset, [gall.ap[0], [2, JH]])
                godd = gout[:, 1:]
                odd = bass.AP(godd.tensor, godd.offset, [godd.ap[0], [2, JH]])
                nc.vector.tensor_tensor_scan(
                    gout[:, :JH], even, odd, 0.0,
                    op0=mybir.AluOpType.add, op1=mybir.AluOpType.add,
                )
                eidx = gspool.tile(
                    [_P, nspad // 16], i16, tag="eidx", name=f"eidx{p}"
                )
                nc.sync.dma_start(
                    eidx[:],
                    eidx_in.ap()[:, p * (nspad // 16) : (p + 1) * (nspad // 16)],
                )
                ex = expool.tile([_P, nspad], fp32, tag="ex", name=f"ex{p}")
                nc.gpsimd.ap_gather(
                    ex[:], gout[:, :JH], eidx[:],
                    channels=_P, num_elems=JH, d=1, num_idxs=nspad,
                )
                nc.vector.tensor_sub(dd[:, :gpc], ex[:, 1 : gpc + 1], ex[:, :gpc])
                pst = pshpool.tile([_P, jt * B], fp32, tag="pst", name="pst")
                for t in range(jt):
                    nc.tensor.matmul(
                        pst[:, t * B : (t + 1) * B],
                        dd[:, t * _P : (t + 1) * _P],
                        sel8[:],
                        start=True, stop=True,
                    )
                nc.vector.tensor_add(sh[:], sh[:], pst[:])

            # software-pipelined emission: gather(p+1) ahead of tail(p)
            emit_table(0)
            gouts = {0: emit_gather(0)}
            for p in range(_NTAB):
                if p + 1 < _NTAB:
                    emit_table(p + 1)
                    gouts[p + 1] = emit_gather(p + 1)
                emit_tail(p, gouts.pop(p))

            shb = perpool.tile([_P, jt * B], bf16, tag="shb")
            nc.vector.tensor_copy(shb[:], sh[:])

            # ---- W1 matmul: accumulate over jt K-tiles --------------------
            n_half = min(512, d1)
            n_banks = -(-d1 // n_half)
            pss = []
            for nb in range(n_banks):
                pst = pswpool.tile([_P, n_half], fp32, tag=f"ps{nb}", name=f"ps{nb}")
                pss.append(pst)
            wgrp = 5 if jt % 5 == 0 else 1  # K-tiles per W1 load
            for jg in range(jt // wgrp):
                w1t = w1pool.tile([_P, wgrp * d1], bf16, tag="w1t")
                nc.sync.dma_start(
                    w1t[:], w1_in.ap()[:, jg * wgrp * d1 : (jg + 1) * wgrp * d1]
                )
                for jl in range(wgrp):
                    j = jg * wgrp + jl
                    lhsT = shb[:, j * B : (j + 1) * B]
                    for nb in range(n_banks):
                        nc.tensor.matmul(
                            pss[nb][:B, :],
                            lhsT,
                            w1t[:, jl * d1 + nb * n_half : jl * d1 + (nb + 1) * n_half],
                            start=(j == 0),
                            stop=(j == jt - 1),
                        )

            h1 = perpool.tile([B, d1], fp32, tag="h1")
            for nb in range(n_banks):
                nc.vector.tensor_copy(
                    h1[:, nb * n_half : (nb + 1) * n_half], pss[nb][:B, :]
                )
            nc.sync.dma_start(h1_out.ap(), h1[:])

    nc.compile()
    return nc


# ---------------------------------------------------------------- host side
def _wrap16(streams):
    """[8, J] per-group streams -> [128, J//16] wrapped-16 layout."""
    ngrp, J = streams.shape
    assert ngrp == 8 and J % 16 == 0
    out = np.zeros((_P, J // 16), streams.dtype)
    for g in range(8):
        out[g * 16 : (g + 1) * 16, :] = streams[g].reshape(J // 16, 16).T
    return out


def _core_slices(snp_ids, node_seg):
    ids = np.asarray(snp_ids).astype(np.int64)
    seg = np.asarray(node_seg).astype(np.int64)
    gpc = N_GENES // N_CORES
    gene_starts = np.searchsorted(seg, np.arange(0, N_GENES + 1, gpc))
    return ids, seg, gpc, gene_starts


def _bucket_counts(ids_c, gene_c, uniq, Kc, gpc):
    """Per-(bucket, gene) even-padded counts. bucket = T*8 + g."""
    cpos = np.searchsorted(uniq, ids_c)
    cchunk = cpos // Kc
    bucketid = (cchunk // 16) * 8 + (cchunk % 8)
    key = bucketid * gpc + gene_c
    cnt = np.bincount(key, minlength=32 * gpc).reshape(32, gpc)
    pad_cnt = cnt + (cnt & 1)
    return cpos, cchunk, bucketid, key, cnt, pad_cnt


def pick_cfg(snp_ids, node_seg):
    """Host pass over the indices: global compact chunk size Kc and padded
    stream length J."""
    ids, seg, gpc, gene_starts = _core_slices(snp_ids, node_seg)
    Kc = 0
    uniqs = []
    for c in range(N_CORES):
        lo, hi = gene_starts[c], gene_starts[c + 1]
        uniq = np.unique(ids[lo:hi])
        uniqs.append(uniq)
        Kc = max(Kc, -(-len(uniq) // (_NCHUNK * 16)) * 16)
    J = 0
    for c in range(N_CORES):
        lo, hi = gene_starts[c], gene_starts[c + 1]
        gene_c = seg[lo:hi] - c * gpc
        _, _, _, _, _, pad_cnt = _bucket_counts(
            ids[lo:hi], gene_c, uniqs[c], Kc, gpc
        )
        J = max(J, 2 + int(pad_cnt.sum(axis=1).max()))
    J = -(-J // 16) * 16
    return Kc, J, uniqs


def prep_inputs(cfg, snp, snp_ids, node_seg, filters, W1, uniqs):
    """Index/metadata preprocessing + zero-padding + pure layout permutation;
    all value computation happens on device."""
    import ml_dtypes

    Kc, J, gpc, gpad = cfg["Kc"], cfg["J"], cfg["gpc"], cfg["gpad"]
    nspad, d1 = cfg["nspad"], cfg["d1"]
    n_cores = cfg["n_cores"]
    ZIDX = 2 * Kc  # zero column (even; pads point here)

    ids, seg, _, gene_starts = _core_slices(snp_ids, node_seg)
    snp = np.asarray(snp, np.float32)
    filters = np.asarray(filters, np.float32)
    W1f = np.asarray(W1, np.float32)

    # mean+replicate routing: prX[m, j] = (1/8) sum_r ft[s(m)X, r, j]
    # ft row q = s*8+r; routeA: s(q) == g(m) = m//16, routeB: s(q) == 8+g(m)
    route = np.zeros((_P, 2 * _P), ml_dtypes.bfloat16)
    for m in range(_P):
        g = m // 16
        route[g * 8 : g * 8 + 8, m] = 1.0 / N_FILT
        route[(8 + g) * 8 : (8 + g) * 8 + 8, _P + m] = 1.0 / N_FILT

    sel8 = np.zeros((_P, 8), ml_dtypes.bfloat16)
    for p in range(_P):
        sel8[p, p % 8] = 1.0

    per_core = []
    for c in range(n_cores):
        lo, hi = gene_starts[c], gene_starts[c + 1]
        ids_c = ids[lo:hi]
        gene_c = seg[lo:hi] - c * gpc
        uniq = uniqs[c]
        nu = len(uniq)
        assert nu <= _NCHUNK * Kc

        # compact value tables (pure permutation of inputs)
        snp_c = np.zeros((B, _NCHUNK * Kc), np.float32)
        snp_c[:, :nu] = snp[:, uniq]
        filt_c = np.zeros((N_FILT, _NCHUNK * Kc), np.float32)
        filt_c[:, :nu] = filters[:, uniq]

        # snp_perm[16g+8h+b, T*Kc+j] = snp_c[b, (16T+g+8h)*Kc+j]
        snp_perm = np.empty((_P, _NTAB * Kc), np.float32)
        filt_perm = np.empty((_P, _NTAB * Kc), np.float32)
        for T in range(_NTAB):
            vi = snp_c[:, 16 * T * Kc : (16 * T + 16) * Kc].reshape(B, 2, 8, Kc)
            snp_perm[:, T * Kc : (T + 1) * Kc] = vi.transpose(2, 1, 0, 3).reshape(
                _P, Kc
            )
            fi = filt_c[:, 16 * T * Kc : (16 * T + 16) * Kc].reshape(
                N_FILT, 16, Kc
            )
            filt_perm[:, T * Kc : (T + 1) * Kc] = fi.transpose(1, 0, 2).reshape(
                _P, Kc
            )
        filt_perm_bf = filt_perm.astype(ml_dtypes.bfloat16)

        cpos, cchunk, bucketid, key, cnt, pad_cnt = _bucket_counts(
            ids_c, gene_c, uniq, Kc, gpc
        )
        clidx = cpos % Kc
        # gene-ordered per-bucket streams with even per-gene padding
        order = np.argsort(bucketid, kind="stable")  # gene order preserved
        skey = key[order]
        stbl = (clidx[order] + np.where((cchunk[order] % 16) >= 8, Kc, 0)).astype(
            np.int64
        )
        flat_cnt = cnt.reshape(-1)
        flat_pad = pad_cnt.reshape(-1)
        starts = np.zeros(32 * gpc, np.int64)  # node start per key
        np.cumsum(flat_cnt[:-1], out=starts[1:])
        offs = np.zeros(32 * gpc, np.int64)  # padded stream offset per key
        pc = flat_pad.reshape(32, gpc)
        row_off = np.cumsum(pc, axis=1)
        offs = (
            2 + np.concatenate([np.zeros((32, 1), np.int64), row_off[:, :-1]], axis=1)
        ).reshape(-1)
        rank = np.arange(len(skey), dtype=np.int64) - starts[skey]
        pos = offs[skey] + rank
        streams = np.full((32, J), ZIDX, np.int16)
        streams[bucketid[order], pos] = stbl.astype(np.int16)
        tot = 2 + pc.sum(axis=1)
        assert int(tot.max()) <= J, f"bucket {int(tot.max())} exceeds J={J}"

        # boundaries (pair units): [0, end(g0), ..., end(g_{gpc-1})], pad
        ends = ((offs.reshape(32, gpc) + pc) // 2 - 1).astype(np.int16)
        ebnd = np.zeros((32, nspad), np.int16)
        ebnd[:, 1 : gpc + 1] = ends
        ebnd[:, gpc + 1 :] = ends[:, -1:]

        gidx_all = np.concatenate(
            [_wrap16(streams[T * 8 : (T + 1) * 8]) for T in range(_NTAB)], axis=1
        )
        eidx_all = np.concatenate(
            [_wrap16(ebnd[T * 8 : (T + 1) * 8]) for T in range(_NTAB)], axis=1
        )

        w1c = np.zeros((gpad, d1), np.float32)
        w1c[:gpc] = W1f[c * gpc : (c + 1) * gpc]
        jt_ = gpad // _P
        w1perm = np.ascontiguousarray(
            w1c.reshape(jt_, _P, d1).transpose(1, 0, 2).reshape(_P, jt_ * d1)
        ).astype(ml_dtypes.bfloat16)

        per_core.append(
            dict(
                snp_perm=snp_perm, filt_perm=filt_perm_bf, sel=sel8, w1c=w1perm,
                mroute=route, gidx=gidx_all, eidx=eidx_all,
            )
        )
    return per_core


def host_tail(h1_sum, b1, g1, be1, W2, b2, g2, be2, W3, b3, g3, be3,
              Wh1, bh1, gh, beh, Wh2, bh2):
    def bn(x, g, be):
        return x * (g / np.sqrt(np.float32(1.0 + BN_EPS))) + be

    relu = lambda x: np.maximum(x, np.float32(0.0))
    h = relu(bn(h1_sum + b1, g1, be1))
    h = relu(bn(h @ W2 + b2, g2, be2))
    feat = relu(bn(h @ W3 + b3, g3, be3))
    m = relu(bn(feat[:, :15] @ Wh1 + bh1, gh, beh))
    return (m @ Wh2 + bh2).astype(np.float32)


_CACHE = {}


def kernel(snp, snp_ids, node_seg, filters, W1, b1, g1, be1, W2, b2, g2, be2,
           W3, b3, g3, be3, Wh1, bh1, gh, beh, Wh2, bh2):
    from concourse import bass_utils

    Kc, J, uniqs = pick_cfg(snp_ids, node_seg)
    cfg = make_cfg(Kc, J)

    key = ("v2", Kc, J)
    if key not in _CACHE:
        _CACHE[key] = build_program(cfg)
    nc = _CACHE[key]

    in_maps = prep_inputs(cfg, snp, snp_ids, node_seg, filters, W1, uniqs)
    res = bass_utils.run_bass_kernel_spmd(
        nc, in_maps, core_ids=list(range(cfg["n_cores"]))
    )
    h1_sum = np.zeros((B, cfg["d1"]), np.float32)
    for c in range(cfg["n_cores"]):
        h1_sum += res.results[c]["h1p"]

    f32 = lambda x: np.asarray(x, np.float32)
    return host_tail(h1_sum, f32(b1), f32(g1), f32(be1), f32(W2), f32(b2),
                     f32(g2), f32(be2), f32(W3), f32(b3), f32(g3), f32(be3),
                     f32(Wh1), f32(bh1), f32(gh), f32(beh), f32(Wh2), f32(bh2))


# revision 3
# speedup vs baseline: 1.1735x; 1.0762x over previous
"""Trainium2 Bass kernel for nn_AgeUGP_v2 (gnn_message_passing).

Reference pipeline:
  snp_h[b,n,f] = snp[b,n] * filters[f,n]
  gathered     = snp_h[:, snp_ids, :]
  per_gene     = segment_sum(gathered, node_seg)   # node_seg sorted
  sample_h     = per_gene.mean(-1)
  h1 = sample_h @ W1 ... tiny MLP tail

Algebraic collapse: the filter axis F is only averaged at the end, so
  sample_h[b,g] = sum_{i in seg g} snp[b, id_i] * fbar[id_i],
  fbar = mean(filters, axis=0).

Device strategy v2 (8 NeuronCores, genes sharded across cores):
  - Per-core SNP COMPACTION: each core's nodes reference ~197k unique SNPs
    (of 500k); the host selects and orders just those (pure permutation),
    split into 64 chunks of Kc.  4 table passes; pass T holds 16 chunks on
    128 partitions: partition p = 16g + 8h + b carries chunk 16T+g+8h,
    batch b.
  - ZERO-JUNK split tables: each partition's gather table is [2*Kc+2] with
    its chunk's values v = snp * fbar at [h*Kc : (h+1)*Kc] and ZEROS
    elsewhere (zeroed once per buffer; DMAs only rewrite data halves).
    An index in [0,Kc) reads chunk A's value on h=0 lanes and exact 0 on
    h=1 lanes (and vice versa), so the 16-lane shared-index junk vanishes
    arithmetically: A/B contributions merge into ONE gene segment.
  - fbar is produced fused on device: a bf16 host-permuted copy of filters
    is hit with 1/8-valued mean+replicate PE matmuls (routeA/routeB) whose
    PSUM output multiplies the table halves on DVE (zeros stay zero).
  - One gpsimd ap_gather per pass streams both chunks' nodes gene-ordered
    (per-gene counts padded to EVEN with pads pointing at the zero column).
    A DVE tensor_tensor_scan with data0/data1 = even/odd stride-2 views
    forms PAIR prefix sums in place (halving scan and extraction size); a
    second ap_gather extracts one prefix per gene END; one adjacent
    difference gives per-(gene,half,batch) sums; a single sel matmul per
    gene tile folds halves+lanes into sample_h [gene, batch] in PSUM.
  - PE matmul with the core's W1 shard (bf16) -> partial h1 [8, 1024];
    host sums the 8 partials and runs the tiny MLP tail (0.01% of FLOPs).
Emission is software-pipelined (gather p+1 ahead of pass-p tail; tables
manually double-buffered so the zero halves persist across passes).
"""

import numpy as np

B = 8
N_SNPS = 500000
N_NODES = 2000000
N_GENES = 20000
N_FILT = 8
N_CORES = 8
BN_EPS = 1e-5

_P = 128
_NCHUNK = 64  # compact SNP chunks per core
_NTAB = 4  # table passes
_EPAD = 16


def make_cfg(Kc, J, n_genes=N_GENES, n_cores=N_CORES, d1=1024):
    gpc = n_genes // n_cores
    jt = -(-gpc // _P)
    gpad = jt * _P
    ns = gpc + 1  # boundaries: dummy zero + one end per gene
    nspad = -(-ns // _EPAD) * _EPAD
    assert J % 16 == 0 and J % 4 == 0
    assert 2 * Kc + 2 <= 2**15, "gather table exceeds num_elems limit"
    assert J <= 32752, "stream length exceeds int16 index range"
    return dict(
        Kc=Kc, J=J, gpc=gpc, gpad=gpad, jt=jt, d1=d1, ns=ns, nspad=nspad,
        n_cores=n_cores,
    )


# ---------------------------------------------------------------- device program
def build_program(cfg):
    import concourse.bass as bass
    import concourse.bacc as bacc
    import concourse.mybir as mybir
    import concourse.tile as tile

    fp32 = mybir.dt.float32
    bf16 = mybir.dt.bfloat16
    i16 = mybir.dt.int16

    Kc, J = cfg["Kc"], cfg["J"]
    jt, d1 = cfg["jt"], cfg["d1"]
    gpc, gpad, nspad = cfg["gpc"], cfg["gpad"], cfg["nspad"]
    TW = 2 * Kc + 2  # table width: [A-half | B-half | zero col pair]
    JH = J // 2

    nc = bacc.Bacc(
        "TRN2", target_bir_lowering=False, debug=False, num_devices=cfg["n_cores"]
    )

    snp_in = nc.dram_tensor("snp_perm", [_P, _NTAB * Kc], fp32, kind="ExternalInput")
    filt_in = nc.dram_tensor("filt_perm", [_P, _NTAB * Kc], bf16, kind="ExternalInput")
    gidx_in = nc.dram_tensor("gidx", [_P, _NTAB * (J // 16)], i16, kind="ExternalInput")
    eidx_in = nc.dram_tensor(
        "eidx", [_P, _NTAB * (nspad // 16)], i16, kind="ExternalInput"
    )
    sel_in = nc.dram_tensor("sel", [_P, 8], bf16, kind="ExternalInput")
    route_in = nc.dram_tensor("mroute", [_P, 2 * _P], bf16, kind="ExternalInput")
    w1_in = nc.dram_tensor("w1c", [_P, jt * d1], bf16, kind="ExternalInput")
    h1_out = nc.dram_tensor("h1p", [B, d1], fp32, kind="ExternalOutput")

    rc = Kc // 8  # route/mul block width (Kc is 16-aligned; rc*8 == Kc)
    assert rc * 8 == Kc and rc <= 512

    with tile.TileContext(nc) as tc:
        with (
            tc.tile_pool(name="per", bufs=1) as perpool,
            tc.tile_pool(name="gs", bufs=2) as gspool,
            tc.tile_pool(name="ft", bufs=2) as ftpool,
            tc.tile_pool(name="ex", bufs=2) as expool,
            tc.tile_pool(name="w1", bufs=2) as w1pool,
            tc.tile_pool(name="ps", bufs=4, space="PSUM") as pspool,
            tc.tile_pool(name="psw", bufs=1, space="PSUM") as pswpool,
            tc.tile_pool(name="psh", bufs=2, space="PSUM") as pshpool,
        ):
            route = perpool.tile([_P, 2 * _P], bf16, tag="route")
            nc.sync.dma_start(route[:], route_in.ap())
            sel8 = perpool.tile([_P, 8], bf16, tag="sel8")
            nc.sync.dma_start(sel8[:], sel_in.ap())

            # sample_h accumulator [gene-tile, (t, b)]
            sh = perpool.tile([_P, jt * B], fp32, tag="sh")
            nc.vector.memset(sh[:], 0.0)
            # dd holds per-(lane,gene) sums; pad cols stay zero forever
            dd = perpool.tile([_P, gpad], bf16, tag="dd")
            nc.vector.memset(dd[:], 0.0)

            # manually double-buffered tables; zero halves persist, DMAs
            # only rewrite data halves each pass (gpsimd memsets: Pool is
            # idle during the lead-in)
            vt = []
            for i in range(2):
                v = perpool.tile([_P, TW], fp32, tag=f"vtab{i}")
                nc.gpsimd.memset(v[:], 0.0)
                vt.append(v)

            def emit_table(T):
                vtab = vt[T % 2]
                ft = ftpool.tile([_P, Kc], bf16, tag="ftl", name=f"ftl{T}")
                nc.sync.dma_start(ft[:], filt_in.ap()[:, T * Kc : (T + 1) * Kc])
                for g in range(8):
                    r0 = 16 * g
                    nc.sync.dma_start(
                        vtab[r0 : r0 + 8, 0:Kc],
                        snp_in.ap()[r0 : r0 + 8, T * Kc : (T + 1) * Kc],
                    )
                    nc.sync.dma_start(
                        vtab[r0 + 8 : r0 + 16, Kc : 2 * Kc],
                        snp_in.ap()[r0 + 8 : r0 + 16, T * Kc : (T + 1) * Kc],
                    )
                for blk in range(8):
                    ks = slice(blk * rc, (blk + 1) * rc)
                    prA = pspool.tile([_P, rc], fp32, tag="pr", name="prA")
                    nc.tensor.matmul(
                        prA[:], route[:, 0:_P], ft[:, ks], start=True, stop=True
                    )
                    nc.vector.tensor_mul(vtab[:, ks], vtab[:, ks], prA[:])
                    prB = pspool.tile([_P, rc], fp32, tag="pr", name="prB")
                    nc.tensor.matmul(
                        prB[:], route[:, _P : 2 * _P], ft[:, ks], start=True, stop=True
                    )
                    ksB = slice(Kc + blk * rc, Kc + (blk + 1) * rc)
                    nc.vector.tensor_mul(vtab[:, ksB], vtab[:, ksB], prB[:])

            def emit_gather(p):
                gidx = gspool.tile([_P, J // 16], i16, tag="gidx", name=f"gidx{p}")
                nc.sync.dma_start(
                    gidx[:], gidx_in.ap()[:, p * (J // 16) : (p + 1) * (J // 16)]
                )
                gout = gspool.tile([_P, J], fp32, tag="gout", name=f"gout{p}")
                nc.gpsimd.ap_gather(
                    gout[:], vt[p % 2][:], gidx[:],
                    channels=_P, num_elems=TW, d=1, num_idxs=J,
                )
                return gout

            def emit_tail(p, gout):
                # pair prefix scan, in place into the first half (writes
                # trail the stride-2 reads)
                gall = gout[:]
                even = bass.AP(gall.tensor, gall.offset, [gall.ap[0], [2, JH]])
                godd = gout[:, 1:]
                odd = bass.AP(godd.tensor, godd.offset, [godd.ap[0], [2, JH]])
                nc.vector.tensor_tensor_scan(
                    gout[:, :JH], even, odd, 0.0,
                    op0=mybir.AluOpType.add, op1=mybir.AluOpType.add,
                )
                eidx = gspool.tile(
                    [_P, nspad // 16], i16, tag="eidx", name=f"eidx{p}"
                )
                nc.sync.dma_start(
                    eidx[:],
                    eidx_in.ap()[:, p * (nspad // 16) : (p + 1) * (nspad // 16)],
                )
                ex = expool.tile([_P, nspad], fp32, tag="ex", name=f"ex{p}")
                nc.gpsimd.ap_gather(
                    ex[:], gout[:, :JH], eidx[:],
                    channels=_P, num_elems=JH, d=1, num_idxs=nspad,
                )
                nc.vector.tensor_sub(dd[:, :gpc], ex[:, 1 : gpc + 1], ex[:, :gpc])
                pst = pshpool.tile([_P, jt * B], fp32, tag="pst", name="pst")
                for t in range(jt):
                    nc.tensor.matmul(
                        pst[:, t * B : (t + 1) * B],
                        dd[:, t * _P : (t + 1) * _P],
                        sel8[:],
                        start=True, stop=True,
                    )
                nc.vector.tensor_add(sh[:], sh[:], pst[:])

            # software-pipelined emission: tables run 2 passes ahead so the
            # next gather's table is built while the current gather runs;
            # gather(p+1) is emitted ahead of tail(p)
            emit_table(0)
            emit_table(1)
            gouts = {0: emit_gather(0)}
            for p in range(_NTAB):
                if p + 2 < _NTAB:
                    emit_table(p + 2)
                if p + 1 < _NTAB:
                    gouts[p + 1] = emit_gather(p + 1)
                emit_tail(p, gouts.pop(p))

            shb = perpool.tile([_P, jt * B], bf16, tag="shb")
            nc.vector.tensor_copy(shb[:], sh[:])

            # ---- W1 matmul: accumulate over jt K-tiles --------------------
            n_half = min(512, d1)
            n_banks = -(-d1 // n_half)
            pss = []
            for nb in range(n_banks):
                pst = pswpool.tile([_P, n_half], fp32, tag=f"ps{nb}", name=f"ps{nb}")
                pss.append(pst)
            wgrp = 5 if jt % 5 == 0 else 1  # K-tiles per W1 load
            for jg in range(jt // wgrp):
                w1t = w1pool.tile([_P, wgrp * d1], bf16, tag="w1t")
                nc.sync.dma_start(
                    w1t[:], w1_in.ap()[:, jg * wgrp * d1 : (jg + 1) * wgrp * d1]
                )
                for jl in range(wgrp):
                    j = jg * wgrp + jl
                    lhsT = shb[:, j * B : (j + 1) * B]
                    for nb in range(n_banks):
                        nc.tensor.matmul(
                            pss[nb][:B, :],
                            lhsT,
                            w1t[:, jl * d1 + nb * n_half : jl * d1 + (nb + 1) * n_half],
                            start=(j == 0),
                            stop=(j == jt - 1),
                        )

            h1 = perpool.tile([B, d1], fp32, tag="h1")
            for nb in range(n_banks):
                nc.vector.tensor_copy(
                    h1[:, nb * n_half : (nb + 1) * n_half], pss[nb][:B, :]
                )
            nc.sync.dma_start(h1_out.ap(), h1[:])

    nc.compile()
    return nc


# ---------------------------------------------------------------- host side
def _wrap16(streams):
    """[8, J] per-group streams -> [128, J//16] wrapped-16 layout."""
    ngrp, J = streams.shape
    assert ngrp == 8 and J % 16 == 0
    out = np.zeros((_P, J // 16), streams.dtype)
    for g in range(8):
        out[g * 16 : (g + 1) * 16, :] = streams[g].reshape(J // 16, 16).T
    return out


def _core_slices(snp_ids, node_seg):
    ids = np.asarray(snp_ids).astype(np.int64)
    seg = np.asarray(node_seg).astype(np.int64)
    gpc = N_GENES // N_CORES
    gene_starts = np.searchsorted(seg, np.arange(0, N_GENES + 1, gpc))
    return ids, seg, gpc, gene_starts


def _bucket_counts(ids_c, gene_c, uniq, Kc, gpc):
    """Per-(bucket, gene) even-padded counts. bucket = T*8 + g."""
    cpos = np.searchsorted(uniq, ids_c)
    cchunk = cpos // Kc
    bucketid = (cchunk // 16) * 8 + (cchunk % 8)
    key = bucketid * gpc + gene_c
    cnt = np.bincount(key, minlength=32 * gpc).reshape(32, gpc)
    pad_cnt = cnt + (cnt & 1)
    return cpos, cchunk, bucketid, key, cnt, pad_cnt


def pick_cfg(snp_ids, node_seg):
    """Host pass over the indices: global compact chunk size Kc and padded
    stream length J."""
    ids, seg, gpc, gene_starts = _core_slices(snp_ids, node_seg)
    Kc = 0
    uniqs = []
    for c in range(N_CORES):
        lo, hi = gene_starts[c], gene_starts[c + 1]
        uniq = np.unique(ids[lo:hi])
        uniqs.append(uniq)
        Kc = max(Kc, -(-len(uniq) // (_NCHUNK * 16)) * 16)
    J = 0
    for c in range(N_CORES):
        lo, hi = gene_starts[c], gene_starts[c + 1]
        gene_c = seg[lo:hi] - c * gpc
        _, _, _, _, _, pad_cnt = _bucket_counts(
            ids[lo:hi], gene_c, uniqs[c], Kc, gpc
        )
        J = max(J, 2 + int(pad_cnt.sum(axis=1).max()))
    J = -(-J // 16) * 16
    return Kc, J, uniqs


def prep_inputs(cfg, snp, snp_ids, node_seg, filters, W1, uniqs):
    """Index/metadata preprocessing + zero-padding + pure layout permutation;
    all value computation happens on device."""
    import ml_dtypes

    Kc, J, gpc, gpad = cfg["Kc"], cfg["J"], cfg["gpc"], cfg["gpad"]
    nspad, d1 = cfg["nspad"], cfg["d1"]
    n_cores = cfg["n_cores"]
    ZIDX = 2 * Kc  # zero column (even; pads point here)

    ids, seg, _, gene_starts = _core_slices(snp_ids, node_seg)
    snp = np.asarray(snp, np.float32)
    filters = np.asarray(filters, np.float32)
    W1f = np.asarray(W1, np.float32)

    # mean+replicate routing: prX[m, j] = (1/8) sum_r ft[s(m)X, r, j]
    # ft row q = s*8+r; routeA: s(q) == g(m) = m//16, routeB: s(q) == 8+g(m)
    route = np.zeros((_P, 2 * _P), ml_dtypes.bfloat16)
    for m in range(_P):
        g = m // 16
        route[g * 8 : g * 8 + 8, m] = 1.0 / N_FILT
        route[(8 + g) * 8 : (8 + g) * 8 + 8, _P + m] = 1.0 / N_FILT

    sel8 = np.zeros((_P, 8), ml_dtypes.bfloat16)
    for p in range(_P):
        sel8[p, p % 8] = 1.0

    per_core = []
    for c in range(n_cores):
        lo, hi = gene_starts[c], gene_starts[c + 1]
        ids_c = ids[lo:hi]
        gene_c = seg[lo:hi] - c * gpc
        uniq = uniqs[c]
        nu = len(uniq)
        assert nu <= _NCHUNK * Kc

        # compact value tables (pure permutation of inputs)
        snp_c = np.zeros((B, _NCHUNK * Kc), np.float32)
        snp_c[:, :nu] = snp[:, uniq]
        filt_c = np.zeros((N_FILT, _NCHUNK * Kc), np.float32)
        filt_c[:, :nu] = filters[:, uniq]

        # snp_perm[16g+8h+b, T*Kc+j] = snp_c[b, (16T+g+8h)*Kc+j]
        snp_perm = np.empty((_P, _NTAB * Kc), np.float32)
        filt_perm = np.empty((_P, _NTAB * Kc), np.float32)
        for T in range(_NTAB):
            vi = snp_c[:, 16 * T * Kc : (16 * T + 16) * Kc].reshape(B, 2, 8, Kc)
            snp_perm[:, T * Kc : (T + 1) * Kc] = vi.transpose(2, 1, 0, 3).reshape(
                _P, Kc
            )
            fi = filt_c[:, 16 * T * Kc : (16 * T + 16) * Kc].reshape(
                N_FILT, 16, Kc
            )
            filt_perm[:, T * Kc : (T + 1) * Kc] = fi.transpose(1, 0, 2).reshape(
                _P, Kc
            )
        filt_perm_bf = filt_perm.astype(ml_dtypes.bfloat16)

        cpos, cchunk, bucketid, key, cnt, pad_cnt = _bucket_counts(
            ids_c, gene_c, uniq, Kc, gpc
        )
        clidx = cpos % Kc
        # gene-ordered per-bucket streams with even per-gene padding
        order = np.argsort(bucketid, kind="stable")  # gene order preserved
        skey = key[order]
        stbl = (clidx[order] + np.where((cchunk[order] % 16) >= 8, Kc, 0)).astype(
            np.int64
        )
        flat_cnt = cnt.reshape(-1)
        flat_pad = pad_cnt.reshape(-1)
        starts = np.zeros(32 * gpc, np.int64)  # node start per key
        np.cumsum(flat_cnt[:-1], out=starts[1:])
        offs = np.zeros(32 * gpc, np.int64)  # padded stream offset per key
        pc = flat_pad.reshape(32, gpc)
        row_off = np.cumsum(pc, axis=1)
        offs = (
            2 + np.concatenate([np.zeros((32, 1), np.int64), row_off[:, :-1]], axis=1)
        ).reshape(-1)
        rank = np.arange(len(skey), dtype=np.int64) - starts[skey]
        pos = offs[skey] + rank
        streams = np.full((32, J), ZIDX, np.int16)
        streams[bucketid[order], pos] = stbl.astype(np.int16)
        tot = 2 + pc.sum(axis=1)
        assert int(tot.max()) <= J, f"bucket {int(tot.max())} exceeds J={J}"

        # boundaries (pair units): [0, end(g0), ..., end(g_{gpc-1})], pad
        ends = ((offs.reshape(32, gpc) + pc) // 2 - 1).astype(np.int16)
        ebnd = np.zeros((32, nspad), np.int16)
        ebnd[:, 1 : gpc + 1] = ends
        ebnd[:, gpc + 1 :] = ends[:, -1:]

        gidx_all = np.concatenate(
            [_wrap16(streams[T * 8 : (T + 1) * 8]) for T in range(_NTAB)], axis=1
        )
        eidx_all = np.concatenate(
            [_wrap16(ebnd[T * 8 : (T + 1) * 8]) for T in range(_NTAB)], axis=1
        )

        w1c = np.zeros((gpad, d1), np.float32)
        w1c[:gpc] = W1f[c * gpc : (c + 1) * gpc]
        jt_ = gpad // _P
        w1perm = np.ascontiguousarray(
            w1c.reshape(jt_, _P, d1).transpose(1, 0, 2).reshape(_P, jt_ * d1)
        ).astype(ml_dtypes.bfloat16)

        per_core.append(
            dict(
                snp_perm=snp_perm, filt_perm=filt_perm_bf, sel=sel8, w1c=w1perm,
                mroute=route, gidx=gidx_all, eidx=eidx_all,
            )
        )
    return per_core


def host_tail(h1_sum, b1, g1, be1, W2, b2, g2, be2, W3, b3, g3, be3,
              Wh1, bh1, gh, beh, Wh2, bh2):
    def bn(x, g, be):
        return x * (g / np.sqrt(np.float32(1.0 + BN_EPS))) + be

    relu = lambda x: np.maximum(x, np.float32(0.0))
    h = relu(bn(h1_sum + b1, g1, be1))
    h = relu(bn(h @ W2 + b2, g2, be2))
    feat = relu(bn(h @ W3 + b3, g3, be3))
    m = relu(bn(feat[:, :15] @ Wh1 + bh1, gh, beh))
    return (m @ Wh2 + bh2).astype(np.float32)


_CACHE = {}


def kernel(snp, snp_ids, node_seg, filters, W1, b1, g1, be1, W2, b2, g2, be2,
           W3, b3, g3, be3, Wh1, bh1, gh, beh, Wh2, bh2):
    from concourse import bass_utils

    Kc, J, uniqs = pick_cfg(snp_ids, node_seg)
    cfg = make_cfg(Kc, J)

    key = ("v2", Kc, J)
    if key not in _CACHE:
        _CACHE[key] = build_program(cfg)
    nc = _CACHE[key]

    in_maps = prep_inputs(cfg, snp, snp_ids, node_seg, filters, W1, uniqs)
    res = bass_utils.run_bass_kernel_spmd(
        nc, in_maps, core_ids=list(range(cfg["n_cores"]))
    )
    h1_sum = np.zeros((B, cfg["d1"]), np.float32)
    for c in range(cfg["n_cores"]):
        h1_sum += res.results[c]["h1p"]

    f32 = lambda x: np.asarray(x, np.float32)
    return host_tail(h1_sum, f32(b1), f32(g1), f32(be1), f32(W2), f32(b2),
                     f32(g2), f32(be2), f32(W3), f32(b3), f32(g3), f32(be3),
                     f32(Wh1), f32(bh1), f32(gh), f32(beh), f32(Wh2), f32(bh2))


# revision 10
# speedup vs baseline: 1.3174x; 1.1226x over previous
"""Trainium2 Bass kernel for nn_AgeUGP_v2 (gnn_message_passing).

Reference pipeline:
  snp_h[b,n,f] = snp[b,n] * filters[f,n]
  gathered     = snp_h[:, snp_ids, :]
  per_gene     = segment_sum(gathered, node_seg)   # node_seg sorted
  sample_h     = per_gene.mean(-1)
  h1 = sample_h @ W1 ... tiny MLP tail

Algebraic collapse: the filter axis F is only averaged at the end, so
  sample_h[b,g] = sum_{i in seg g} snp[b, id_i] * fbar[id_i],
  fbar = mean(filters, axis=0).

Device strategy v2 (8 NeuronCores, genes sharded across cores):
  - Per-core SNP COMPACTION: each core's nodes reference ~197k unique SNPs
    (of 500k); the host selects and orders just those (pure permutation),
    split into 64 chunks of Kc.  4 table passes; pass T holds 16 chunks on
    128 partitions: partition p = 16g + 8h + b carries chunk 16T+g+8h,
    batch b.
  - ZERO-JUNK split tables: each partition's gather table is [2*Kc+2] with
    its chunk's values v = snp * fbar at [h*Kc : (h+1)*Kc] and ZEROS
    elsewhere (zeroed once per buffer; DMAs only rewrite data halves).
    An index in [0,Kc) reads chunk A's value on h=0 lanes and exact 0 on
    h=1 lanes (and vice versa), so the 16-lane shared-index junk vanishes
    arithmetically: A/B contributions merge into ONE gene segment.
  - fbar is produced fused on device: a bf16 host-permuted copy of filters
    is hit with 1/8-valued mean+replicate PE matmuls (routeA/routeB) whose
    PSUM output multiplies the table halves on DVE (zeros stay zero).
  - One gpsimd ap_gather per pass streams both chunks' nodes gene-ordered
    (per-gene counts padded to EVEN with pads pointing at the zero column).
    A DVE tensor_tensor_scan with data0/data1 = even/odd stride-2 views
    forms PAIR prefix sums in place (halving scan and extraction size); a
    second ap_gather extracts one prefix per gene END; one adjacent
    difference gives per-(gene,half,batch) sums; a single sel matmul per
    gene tile folds halves+lanes into sample_h [gene, batch] in PSUM.
  - PE matmul with the core's W1 shard (bf16) -> partial h1 [8, 1024];
    host sums the 8 partials and runs the tiny MLP tail (0.01% of FLOPs).
Emission is software-pipelined (gather p+1 ahead of pass-p tail; tables
manually double-buffered so the zero halves persist across passes).
"""

import numpy as np

B = 8
N_SNPS = 500000
N_NODES = 2000000
N_GENES = 20000
N_FILT = 8
N_CORES = 8
BN_EPS = 1e-5

_P = 128
_NCHUNK = 64  # compact SNP chunks per core
_NTAB = 4  # table passes
_EPAD = 16


def make_cfg(Kc, J, n_genes=N_GENES, n_cores=N_CORES, d1=1024):
    gpc = n_genes // n_cores
    jt = -(-gpc // _P)
    gpad = jt * _P
    ns = gpc + 1  # boundaries: dummy zero + one end per gene
    nspad = -(-ns // _EPAD) * _EPAD
    assert J % 16 == 0 and J % 4 == 0
    assert 2 * Kc + 2 <= 2**15, "gather table exceeds num_elems limit"
    assert J <= 32752, "stream length exceeds int16 index range"
    return dict(
        Kc=Kc, J=J, gpc=gpc, gpad=gpad, jt=jt, d1=d1, ns=ns, nspad=nspad,
        n_cores=n_cores,
    )


# ---------------------------------------------------------------- device program
def build_program(cfg):
    import concourse.bass as bass
    import concourse.bacc as bacc
    import concourse.mybir as mybir
    import concourse.tile as tile

    fp32 = mybir.dt.float32
    bf16 = mybir.dt.bfloat16
    i16 = mybir.dt.int16

    Kc, J = cfg["Kc"], cfg["J"]
    jt, d1 = cfg["jt"], cfg["d1"]
    gpc, gpad, nspad = cfg["gpc"], cfg["gpad"], cfg["nspad"]
    TW = 2 * Kc + 2  # table width: [A-half | B-half | zero col pair]
    JH = J // 2

    nc = bacc.Bacc(
        "TRN2", target_bir_lowering=False, debug=False, num_devices=cfg["n_cores"]
    )

    snp_in = nc.dram_tensor("snp_perm", [_P, _NTAB * TW], fp32, kind="ExternalInput")
    filt_in = nc.dram_tensor("filt_perm", [_P, _NTAB * Kc], bf16, kind="ExternalInput")
    gidx_in = nc.dram_tensor("gidx", [_P, _NTAB * (J // 16)], i16, kind="ExternalInput")
    eidx_in = nc.dram_tensor(
        "eidx", [_P, _NTAB * (nspad // 16)], i16, kind="ExternalInput"
    )
    sel_in = nc.dram_tensor("sel", [_P, 8], bf16, kind="ExternalInput")
    route_in = nc.dram_tensor("mroute", [_P, 2 * _P], bf16, kind="ExternalInput")
    w1_in = nc.dram_tensor("w1c", [_P, jt * d1], bf16, kind="ExternalInput")
    h1_out = nc.dram_tensor("h1p", [B, d1], fp32, kind="ExternalOutput")

    rc = Kc // 4  # route/mul block width (2-bank PSUM tiles)
    assert rc * 4 == Kc and rc * 4 <= 4096

    with tile.TileContext(nc) as tc:
        with (
            tc.tile_pool(name="per", bufs=1) as perpool,
            tc.tile_pool(name="tab", bufs=2) as tabpool,
            tc.tile_pool(name="gs", bufs=2) as gspool,
            tc.tile_pool(name="ft", bufs=2) as ftpool,
            tc.tile_pool(name="ex", bufs=2) as expool,
            tc.tile_pool(name="w1", bufs=2) as w1pool,
            tc.tile_pool(name="ps", bufs=2, space="PSUM") as pspool,
            tc.tile_pool(name="psw", bufs=1, space="PSUM") as pswpool,
            tc.tile_pool(name="psh", bufs=2, space="PSUM") as pshpool,
        ):
            route = perpool.tile([_P, 2 * _P], bf16, tag="route")
            nc.sync.dma_start(route[:], route_in.ap())
            sel8 = perpool.tile([_P, 8], bf16, tag="sel8")
            nc.sync.dma_start(sel8[:], sel_in.ap())

            # sample_h accumulator [gene-tile, (t, b)]
            sh = perpool.tile([_P, jt * B], fp32, tag="sh")
            nc.vector.memset(sh[:], 0.0)
            # dd holds per-(lane,gene) sums; pad cols stay zero forever
            dd = perpool.tile([_P, gpad], bf16, tag="dd")
            nc.vector.memset(dd[:], 0.0)

            vtabs = {}

            def emit_table(T):
                # DRAM rows carry the zero-split layout already (data half +
                # zero half per partition parity): two wide DMAs per pass,
                # the zero columns arrive as part of the transfer
                vtab = tabpool.tile([_P, TW], fp32, tag="vtab", name=f"vtab{T}")
                ft = ftpool.tile([_P, Kc], bf16, tag="ftl", name=f"ftl{T}")
                nc.sync.dma_start(ft[:], filt_in.ap()[:, T * Kc : (T + 1) * Kc])
                nc.sync.dma_start(
                    vtab[:, 0:Kc], snp_in.ap()[:, T * TW : T * TW + Kc]
                )
                nc.sync.dma_start(
                    vtab[:, Kc:TW], snp_in.ap()[:, T * TW + Kc : (T + 1) * TW]
                )
                for half in range(2):
                    for blk in range(4):
                        pr = pspool.tile([_P, rc], fp32, tag="pr", name="pr")
                        nc.tensor.matmul(
                            pr[:],
                            route[:, half * _P : (half + 1) * _P],
                            ft[:, blk * rc : (blk + 1) * rc],
                            start=True, stop=True,
                        )
                        ks = slice(half * Kc + blk * rc, half * Kc + (blk + 1) * rc)
                        nc.vector.tensor_mul(vtab[:, ks], vtab[:, ks], pr[:])
                vtabs[T] = vtab

            def emit_gather(p):
                gidx = gspool.tile([_P, J // 16], i16, tag="gidx", name=f"gidx{p}")
                nc.sync.dma_start(
                    gidx[:], gidx_in.ap()[:, p * (J // 16) : (p + 1) * (J // 16)]
                )
                gout = gspool.tile([_P, J], fp32, tag="gout", name=f"gout{p}")
                nc.gpsimd.ap_gather(
                    gout[:], vtabs.pop(p)[:], gidx[:],
                    channels=_P, num_elems=TW, d=1, num_idxs=J,
                )
                return gout

            def emit_tail(p, gout):
                # pair prefix scan, in place into the first half (writes
                # trail the stride-2 reads)
                gall = gout[:]
                even = bass.AP(gall.tensor, gall.offset, [gall.ap[0], [2, JH]])
                godd = gout[:, 1:]
                odd = bass.AP(godd.tensor, godd.offset, [godd.ap[0], [2, JH]])
                nc.vector.tensor_tensor_scan(
                    gout[:, :JH], even, odd, 0.0,
                    op0=mybir.AluOpType.add, op1=mybir.AluOpType.add,
                )
                eidx = gspool.tile(
                    [_P, nspad // 16], i16, tag="eidx", name=f"eidx{p}"
                )
                nc.sync.dma_start(
                    eidx[:],
                    eidx_in.ap()[:, p * (nspad // 16) : (p + 1) * (nspad // 16)],
                )
                ex = expool.tile([_P, nspad], fp32, tag="ex", name=f"ex{p}")
                nc.gpsimd.ap_gather(
                    ex[:], gout[:, :JH], eidx[:],
                    channels=_P, num_elems=JH, d=1, num_idxs=nspad,
                )
                nc.vector.tensor_sub(dd[:, :gpc], ex[:, 1 : gpc + 1], ex[:, :gpc])
                pst = pshpool.tile([_P, jt * B], fp32, tag="pst", name="pst")
                for t in range(jt):
                    nc.tensor.matmul(
                        pst[:, t * B : (t + 1) * B],
                        dd[:, t * _P : (t + 1) * _P],
                        sel8[:],
                        start=True, stop=True,
                    )
                nc.vector.tensor_add(sh[:], sh[:], pst[:])

            # software-pipelined emission: tables run 2 passes ahead so the
            # next gather's table is built while the current gather runs;
            # gather(p+1) is emitted ahead of tail(p)
            emit_table(0)
            emit_table(1)
            gouts = {0: emit_gather(0)}
            for p in range(_NTAB):
                if p + 2 < _NTAB:
                    emit_table(p + 2)
                if p + 1 < _NTAB:
                    gouts[p + 1] = emit_gather(p + 1)
                emit_tail(p, gouts.pop(p))

            shb = perpool.tile([_P, jt * B], bf16, tag="shb")
            nc.vector.tensor_copy(shb[:], sh[:])

            # ---- W1 matmul: accumulate over jt K-tiles --------------------
            n_half = min(512, d1)
            n_banks = -(-d1 // n_half)
            pss = []
            for nb in range(n_banks):
                pst = pswpool.tile([_P, n_half], fp32, tag=f"ps{nb}", name=f"ps{nb}")
                pss.append(pst)
            wgrp = 5 if jt % 5 == 0 else 1  # K-tiles per W1 load
            for jg in range(jt // wgrp):
                w1t = w1pool.tile([_P, wgrp * d1], bf16, tag="w1t")
                nc.sync.dma_start(
                    w1t[:], w1_in.ap()[:, jg * wgrp * d1 : (jg + 1) * wgrp * d1]
                )
                for jl in range(wgrp):
                    j = jg * wgrp + jl
                    lhsT = shb[:, j * B : (j + 1) * B]
                    for nb in range(n_banks):
                        nc.tensor.matmul(
                            pss[nb][:B, :],
                            lhsT,
                            w1t[:, jl * d1 + nb * n_half : jl * d1 + (nb + 1) * n_half],
                            start=(j == 0),
                            stop=(j == jt - 1),
                        )

            h1 = perpool.tile([B, d1], fp32, tag="h1")
            for nb in range(n_banks):
                nc.vector.tensor_copy(
                    h1[:, nb * n_half : (nb + 1) * n_half], pss[nb][:B, :]
                )
            nc.sync.dma_start(h1_out.ap(), h1[:])

    nc.compile()
    return nc


# ---------------------------------------------------------------- host side
def _wrap16(streams):
    """[8, J] per-group streams -> [128, J//16] wrapped-16 layout."""
    ngrp, J = streams.shape
    assert ngrp == 8 and J % 16 == 0
    out = np.zeros((_P, J // 16), streams.dtype)
    for g in range(8):
        out[g * 16 : (g + 1) * 16, :] = streams[g].reshape(J // 16, 16).T
    return out


def _core_slices(snp_ids, node_seg):
    ids = np.asarray(snp_ids).astype(np.int64)
    seg = np.asarray(node_seg).astype(np.int64)
    gpc = N_GENES // N_CORES
    gene_starts = np.searchsorted(seg, np.arange(0, N_GENES + 1, gpc))
    return ids, seg, gpc, gene_starts


def _bucket_counts(ids_c, gene_c, uniq, Kc, gpc):
    """Per-(bucket, gene) even-padded counts. bucket = T*8 + g."""
    cpos = np.searchsorted(uniq, ids_c)
    cchunk = cpos // Kc
    bucketid = (cchunk // 16) * 8 + (cchunk % 8)
    key = bucketid * gpc + gene_c
    cnt = np.bincount(key, minlength=32 * gpc).reshape(32, gpc)
    pad_cnt = cnt + (cnt & 1)
    return cpos, cchunk, bucketid, key, cnt, pad_cnt


def pick_cfg(snp_ids, node_seg):
    """Host pass over the indices: global compact chunk size Kc and padded
    stream length J."""
    ids, seg, gpc, gene_starts = _core_slices(snp_ids, node_seg)
    Kc = 0
    uniqs = []
    for c in range(N_CORES):
        lo, hi = gene_starts[c], gene_starts[c + 1]
        uniq = np.unique(ids[lo:hi])
        uniqs.append(uniq)
        Kc = max(Kc, -(-len(uniq) // (_NCHUNK * 16)) * 16)
    J = 0
    for c in range(N_CORES):
        lo, hi = gene_starts[c], gene_starts[c + 1]
        gene_c = seg[lo:hi] - c * gpc
        _, _, _, _, _, pad_cnt = _bucket_counts(
            ids[lo:hi], gene_c, uniqs[c], Kc, gpc
        )
        J = max(J, 2 + int(pad_cnt.sum(axis=1).max()))
    J = -(-J // 16) * 16
    return Kc, J, uniqs


def prep_inputs(cfg, snp, snp_ids, node_seg, filters, W1, uniqs):
    """Index/metadata preprocessing + zero-padding + pure layout permutation;
    all value computation happens on device."""
    import ml_dtypes

    Kc, J, gpc, gpad = cfg["Kc"], cfg["J"], cfg["gpc"], cfg["gpad"]
    nspad, d1 = cfg["nspad"], cfg["d1"]
    n_cores = cfg["n_cores"]
    ZIDX = 2 * Kc  # zero column (even; pads point here)

    ids, seg, _, gene_starts = _core_slices(snp_ids, node_seg)
    snp = np.asarray(snp, np.float32)
    filters = np.asarray(filters, np.float32)
    W1f = np.asarray(W1, np.float32)

    # mean+replicate routing: prX[m, j] = (1/8) sum_r ft[s(m)X, r, j]
    # ft row q = s*8+r; routeA: s(q) == g(m) = m//16, routeB: s(q) == 8+g(m)
    route = np.zeros((_P, 2 * _P), ml_dtypes.bfloat16)
    for m in range(_P):
        g = m // 16
        route[g * 8 : g * 8 + 8, m] = 1.0 / N_FILT
        route[(8 + g) * 8 : (8 + g) * 8 + 8, _P + m] = 1.0 / N_FILT

    sel8 = np.zeros((_P, 8), ml_dtypes.bfloat16)
    for p in range(_P):
        sel8[p, p % 8] = 1.0

    per_core = []
    for c in range(n_cores):
        lo, hi = gene_starts[c], gene_starts[c + 1]
        ids_c = ids[lo:hi]
        gene_c = seg[lo:hi] - c * gpc
        uniq = uniqs[c]
        nu = len(uniq)
        assert nu <= _NCHUNK * Kc

        # compact value tables (pure permutation of inputs)
        snp_c = np.zeros((B, _NCHUNK * Kc), np.float32)
        snp_c[:, :nu] = snp[:, uniq]
        filt_c = np.zeros((N_FILT, _NCHUNK * Kc), np.float32)
        filt_c[:, :nu] = filters[:, uniq]

        # zero-split table layout, pre-laid in DRAM (pure permutation + zero
        # padding): row p = 16g+8h+b, pass-T block cols [h*Kc : (h+1)*Kc]
        # hold snp_c[b, (16T+g+8h)*Kc + j]; the other half and the trailing
        # zero column pair stay 0
        TW = 2 * Kc + 2
        snp_perm = np.zeros((_P, _NTAB * TW), np.float32)
        sp4 = snp_perm.reshape(8, 2, 8, _NTAB * TW)  # [g, h, b, cols]
        filt_perm = np.empty((_P, _NTAB * Kc), np.float32)
        for T in range(_NTAB):
            vi = snp_c[:, 16 * T * Kc : (16 * T + 16) * Kc].reshape(B, 2, 8, Kc)
            perm = vi.transpose(2, 1, 0, 3)  # [g, h, b, j]
            for h in range(2):
                sp4[:, h, :, T * TW + h * Kc : T * TW + (h + 1) * Kc] = perm[:, h]
            fi = filt_c[:, 16 * T * Kc : (16 * T + 16) * Kc].reshape(
                N_FILT, 16, Kc
            )
            filt_perm[:, T * Kc : (T + 1) * Kc] = fi.transpose(1, 0, 2).reshape(
                _P, Kc
            )
        filt_perm_bf = filt_perm.astype(ml_dtypes.bfloat16)

        cpos, cchunk, bucketid, key, cnt, pad_cnt = _bucket_counts(
            ids_c, gene_c, uniq, Kc, gpc
        )
        clidx = cpos % Kc
        # gene-ordered per-bucket streams with even per-gene padding
        order = np.argsort(bucketid, kind="stable")  # gene order preserved
        skey = key[order]
        stbl = (clidx[order] + np.where((cchunk[order] % 16) >= 8, Kc, 0)).astype(
            np.int64
        )
        flat_cnt = cnt.reshape(-1)
        flat_pad = pad_cnt.reshape(-1)
        starts = np.zeros(32 * gpc, np.int64)  # node start per key
        np.cumsum(flat_cnt[:-1], out=starts[1:])
        offs = np.zeros(32 * gpc, np.int64)  # padded stream offset per key
        pc = flat_pad.reshape(32, gpc)
        row_off = np.cumsum(pc, axis=1)
        offs = (
            2 + np.concatenate([np.zeros((32, 1), np.int64), row_off[:, :-1]], axis=1)
        ).reshape(-1)
        rank = np.arange(len(skey), dtype=np.int64) - starts[skey]
        pos = offs[skey] + rank
        streams = np.full((32, J), ZIDX, np.int16)
        streams[bucketid[order], pos] = stbl.astype(np.int16)
        tot = 2 + pc.sum(axis=1)
        assert int(tot.max()) <= J, f"bucket {int(tot.max())} exceeds J={J}"

        # boundaries (pair units): [0, end(g0), ..., end(g_{gpc-1})], pad
        ends = ((offs.reshape(32, gpc) + pc) // 2 - 1).astype(np.int16)
        ebnd = np.zeros((32, nspad), np.int16)
        ebnd[:, 1 : gpc + 1] = ends
        ebnd[:, gpc + 1 :] = ends[:, -1:]

        gidx_all = np.concatenate(
            [_wrap16(streams[T * 8 : (T + 1) * 8]) for T in range(_NTAB)], axis=1
        )
        eidx_all = np.concatenate(
            [_wrap16(ebnd[T * 8 : (T + 1) * 8]) for T in range(_NTAB)], axis=1
        )

        w1c = np.zeros((gpad, d1), np.float32)
        w1c[:gpc] = W1f[c * gpc : (c + 1) * gpc]
        jt_ = gpad // _P
        w1perm = np.ascontiguousarray(
            w1c.reshape(jt_, _P, d1).transpose(1, 0, 2).reshape(_P, jt_ * d1)
        ).astype(ml_dtypes.bfloat16)

        per_core.append(
            dict(
                snp_perm=snp_perm, filt_perm=filt_perm_bf, sel=sel8, w1c=w1perm,
                mroute=route, gidx=gidx_all, eidx=eidx_all,
            )
        )
    return per_core


def host_tail(h1_sum, b1, g1, be1, W2, b2, g2, be2, W3, b3, g3, be3,
              Wh1, bh1, gh, beh, Wh2, bh2):
    def bn(x, g, be):
        return x * (g / np.sqrt(np.float32(1.0 + BN_EPS))) + be

    relu = lambda x: np.maximum(x, np.float32(0.0))
    h = relu(bn(h1_sum + b1, g1, be1))
    h = relu(bn(h @ W2 + b2, g2, be2))
    feat = relu(bn(h @ W3 + b3, g3, be3))
    m = relu(bn(feat[:, :15] @ Wh1 + bh1, gh, beh))
    return (m @ Wh2 + bh2).astype(np.float32)


_CACHE = {}


def kernel(snp, snp_ids, node_seg, filters, W1, b1, g1, be1, W2, b2, g2, be2,
           W3, b3, g3, be3, Wh1, bh1, gh, beh, Wh2, bh2):
    from concourse import bass_utils

    Kc, J, uniqs = pick_cfg(snp_ids, node_seg)
    cfg = make_cfg(Kc, J)

    key = ("v2", Kc, J)
    if key not in _CACHE:
        _CACHE[key] = build_program(cfg)
    nc = _CACHE[key]

    in_maps = prep_inputs(cfg, snp, snp_ids, node_seg, filters, W1, uniqs)
    res = bass_utils.run_bass_kernel_spmd(
        nc, in_maps, core_ids=list(range(cfg["n_cores"]))
    )
    h1_sum = np.zeros((B, cfg["d1"]), np.float32)
    for c in range(cfg["n_cores"]):
        h1_sum += res.results[c]["h1p"]

    f32 = lambda x: np.asarray(x, np.float32)
    return host_tail(h1_sum, f32(b1), f32(g1), f32(be1), f32(W2), f32(b2),
                     f32(g2), f32(be2), f32(W3), f32(b3), f32(g3), f32(be3),
                     f32(Wh1), f32(bh1), f32(gh), f32(beh), f32(Wh2), f32(bh2))
